# revision 27
# baseline (speedup 1.0000x reference)
"""Trainium2 Bass kernel for nn_MACAM (cross-attn modulation + instance norm).

Strategy: pure data parallel - batch B=16 sharded 2 samples per core over 8
NeuronCores.  Per sample the conv + fc_k are folded on the host into a single
matrix Mf = fc_k_w.T @ conv_w so the device computes
    kcT = Mf.T @ ws.T + c0,   attn = kcT.T @ h + kb        (kb host-folded)
The argmax/gather runs on-chip as a one-hot matmul.  The beta/gamma map
matmuls run with K=64 (attn lives on partitions 0-63), bf16 weights and
moving operand.  InstanceNorm scale `s` folds at piece level via
scalar_tensor_tensor  tmp = (gamma_map * s) * h  on DVE, and the final add
runs as a PE identity-accumulate into the beta-map PSUM (beta2 =
beta + t2*gamma folded on-chip).  Output is written bf16 and upcast on the
host.  attn pieces are interleaved with modulation pieces so the PE never
idles long enough for HAM to re-throttle the clock.
"""

import os
import sys

os.environ.setdefault("MYCRO_LOCAL_CACHE", "1")
sys.path.insert(0, "/opt/trn_rl_repo")

import numpy as np

import concourse.bacc as bacc
import concourse.bass as bass
import concourse.mybir as mybir
import concourse.tile as tile
from concourse.bass_utils import run_bass_kernel_spmd

N_CORES = 8
B, C, H, W = 16, 512, 64, 64
HW = H * W
L, D, Q = 64, 512, 512
S = B // N_CORES          # samples per core
EPS = 1e-5
NP = 8                    # HW pieces of 512
NC4 = 4                   # channel chunks of 128

f32 = mybir.dt.float32
f32r = mybir.dt.float32r
bf16 = mybir.dt.bfloat16
AF = mybir.ActivationFunctionType
ALU = mybir.AluOpType
AX = mybir.AxisListType


def _build_program():
    nc = bacc.Bacc("TRN2", target_bir_lowering=False, debug=False,
                   num_devices=N_CORES)
    dt_ = nc.dram_tensor
    h_d = dt_("h", [S, C, H, W], f32, kind="ExternalInput").ap()
    wsT_d = dt_("ws_t", [S, D, L], f32, kind="ExternalInput").ap()
    wtT_d = dt_("wt_t", [S, D, L], f32, kind="ExternalInput").ap()
    wt_d = dt_("wt_bf", [S, L, D], bf16, kind="ExternalInput").ap()
    kb_d = dt_("kb_col", [S, L, 1], f32, kind="ExternalInput").ap()
    mf_d = dt_("mf", [D, C], f32, kind="ExternalInput").ap()
    c0_d = dt_("c0_4", [4, 128], f32, kind="ExternalInput").ap()
    fw_d = dt_("fc_w_bf", [D, 2 * C], bf16, kind="ExternalInput").ap()
    fb_d = dt_("fc_b_row", [1, 2 * C], f32, kind="ExternalInput").ap()
    inw_d = dt_("in_w4", [4, 128], f32, kind="ExternalInput").ap()
    inb_d = dt_("in_b4", [4, 128], f32, kind="ExternalInput").ap()
    id64_d = dt_("identity64_bf", [L, L], bf16, kind="ExternalInput").ap()
    nid64_d = dt_("negid64", [L, L], f32, kind="ExternalInput").ap()
    ones_d = dt_("ones64", [L, L], f32, kind="ExternalInput").ap()
    epz_d = dt_("eps_zero", [2, 128], f32, kind="ExternalInput").ap()
    id128_d = dt_("identity128", [128, 128], f32, kind="ExternalInput").ap()
    id128b_d = dt_("identity128_bf", [128, 128], bf16, kind="ExternalInput").ap()
    sel8_d = dt_("sel8", [8, 8 * L], f32, kind="ExternalInput").ap()
    out_d = dt_("out", [S, C, HW], bf16, kind="ExternalOutput").ap()

    h_v = h_d.rearrange("s (n p) a b -> s n p (a b)", p=128)     # [S,4,128,4096]
    out_v = out_d.rearrange("s (n p) q -> s n p q", p=128)
    wsT_v = wsT_d.rearrange("s (n p) l -> s p n l", p=128)       # [S,128,4,64]
    wtT_v = wtT_d.rearrange("s (n p) l -> s p n l", p=128)

    with tile.TileContext(nc) as tc:
        with (
            tc.tile_pool(name="wpool", bufs=1) as wpool,
            tc.tile_pool(name="hpool", bufs=8) as hpool,
            tc.tile_pool(name="attnpool", bufs=2) as attnpool,
            tc.tile_pool(name="spool", bufs=2) as spool,
            tc.tile_pool(name="piece", bufs=3) as piece,
            tc.tile_pool(name="ps_bg", bufs=1, space="PSUM") as ps_bg,
            tc.tile_pool(name="ps_ring", bufs=7, space="PSUM") as ps_ring,
        ):
            # ---- persistent weights ----
            mf_t = []     # Mf chunks [128(d), 512(c)] fp32
            for j in range(4):
                t = wpool.tile([128, C], f32, tag=f"mf{j}")
                nc.scalar.dma_start(t[:], mf_d[j * 128:(j + 1) * 128, :])
                mf_t.append(t)
            fw_t = []     # fc_w.T chunks [128(d), 1024(j)] bf16
            for j in range(4):
                t = wpool.tile([128, 2 * C], bf16, tag=f"fw{j}")
                nc.scalar.dma_start(t[:], fw_d[j * 128:(j + 1) * 128, :])
                fw_t.append(t)
            c0_col = wpool.tile([128, 4], f32, tag="c0")
            nc.gpsimd.dma_start(c0_col[:], c0_d.rearrange("j p -> p j"))
            inw_col = wpool.tile([128, 4], f32, tag="inw")
            nc.gpsimd.dma_start(inw_col[:], inw_d.rearrange("j p -> p j"))
            inb_col = wpool.tile([128, 4], f32, tag="inb")
            nc.gpsimd.dma_start(inb_col[:], inb_d.rearrange("j p -> p j"))
            fcb_row = wpool.tile([1, 2 * C], f32r, tag="fcb")
            nc.gpsimd.dma_start(fcb_row[:], fb_d.bitcast(f32r))
            id64b = wpool.tile([L, L], bf16, tag="id64b")
            nc.gpsimd.dma_start(id64b[:], id64_d)
            nid64 = wpool.tile([L, L], f32, tag="nid64")
            nc.gpsimd.dma_start(nid64[:], nid64_d)
            ones64 = wpool.tile([L, L], f32, tag="ones64")
            nc.gpsimd.dma_start(ones64[:], ones_d)
            ones1r = wpool.tile([1, L], f32r, tag="ones1r")
            nc.gpsimd.dma_start(ones1r[:], ones_d[0:1, :].bitcast(f32r))
            epz = wpool.tile([128, 2], f32, tag="epz")
            nc.gpsimd.dma_start(epz[:], epz_d.rearrange("j p -> p j"))
            id128b = wpool.tile([128, 128], bf16, tag="id128b")
            nc.gpsimd.dma_start(id128b[:], id128b_d)
            id128r = wpool.tile([128, 128], f32r, tag="id128r")
            nc.gpsimd.dma_start(id128r[:], id128_d.bitcast(f32r))
            id128 = wpool.tile([128, 128], f32, tag="id128")
            nc.gpsimd.dma_start(id128[:], id128_d)
            sel8 = wpool.tile([8, 8 * L], f32r, tag="sel8")
            nc.gpsimd.dma_start(sel8[:], sel8_d.bitcast(f32r))

            # ---- all per-sample input DMAs up front ----
            wsT4s, wtT4s, wts, kbs, hts = [], [], [], [], []
            for s in range(S):
                wsT4 = spool.tile([128, 4 * L], f32, tag="wsT4")
                nc.sync.dma_start(
                    wsT4[:].rearrange("p (n l) -> p n l", l=L), wsT_v[s])
                wtT4 = spool.tile([128, 4 * L], f32, tag="wtT4")
                nc.sync.dma_start(
                    wtT4[:].rearrange("p (n l) -> p n l", l=L), wtT_v[s])
                wt_sb = spool.tile([L, D], bf16, tag="wt_sb")
                nc.sync.dma_start(wt_sb[:], wt_d[s])
                kb_col = spool.tile([L, 1], f32, tag="kb")
                nc.sync.dma_start(kb_col[:], kb_d[s])
                wsT4s.append(wsT4); wtT4s.append(wtT4)
                wts.append(wt_sb); kbs.append(kb_col)
            for s in range(S):
                h_t = []
                for cc in range(NC4):
                    t = hpool.tile([128, HW], f32r, tag="h")
                    nc.sync.dma_start(t[:], h_v[s, cc].bitcast(f32r))
                    h_t.append(t)
                hts.append(h_t)

            st = [dict() for _ in range(S)]

            def prologue_scores(s):
                wsT4, wtT4, wt_sb = wsT4s[s], wtT4s[s], wts[s]
                # one PSUM bank holds the whole small path as scratch:
                #   [0:64, 0:64]    scores       [0:1, 64:128] colsum
                #   [0:64,128:192]  left         [0:64,192:224] PT (bf16)
                #   [0:8, 224:352]  stT          [0:128,384:448] kcT (4x)
                #   [0:128,448:512] waT (4x)
                # then gamma -> [0:64, 0:512], copied out, then beta.
                scr = ps_bg.tile([128, 512], f32, tag="bg")
                st[s]["scr"] = scr

                scores_ps = scr[0:L, 0:L]
                for j in range(4):
                    nc.tensor.matmul(
                        scores_ps,
                        wsT4[:, j * L:(j + 1) * L], wtT4[:, j * L:(j + 1) * L],
                        start=(j == 0), stop=(j == 3))
                scores_sb = spool.tile([L, L], f32, tag="scores_sb")
                nc.scalar.copy(scores_sb[:], scores_ps)
                colsum_ps = scr[0:1, 64:64 + L]
                nc.tensor.matmul(colsum_ps, ones64[:, 0:1], scores_sb[:],
                                 start=True, stop=True)
                colsum_row = spool.tile([1, L], f32, tag="colsum")
                nc.scalar.copy(colsum_row[:], colsum_ps)
                left_ps = scr[0:L, 128:128 + L]
                nc.tensor.matmul(left_ps, ones64[0:1, :], colsum_row[:],
                                 start=True, stop=False)
                nc.tensor.matmul(left_ps, nid64[:], scores_sb[:],
                                 start=False, stop=True)
                rowmax = spool.tile([L, 1], f32, tag="rowmax")
                nc.vector.tensor_reduce(rowmax[:], left_ps, AX.X, ALU.max)
                P_sb = spool.tile([L, L], bf16, tag="P_sb")
                nc.vector.tensor_scalar(P_sb[:], left_ps, rowmax[:], None,
                                        ALU.is_equal)
                PT_ps = scr[0:L, 192:224].bitcast(bf16)
                nc.tensor.transpose(PT_ps, P_sb[:], id64b[:])
                PT_sb = spool.tile([L, L], bf16, tag="PT_sb")
                nc.scalar.copy(PT_sb[:], PT_ps)

                # kcT = Mf.T @ ws.T + c0
                kcT_sb = spool.tile([128, L * NC4], f32r, tag="kcT_sb")
                st[s]["kcT"] = kcT_sb
                for cc in range(NC4):
                    kcT_ps = scr[0:128, 384:384 + L]
                    for j in range(4):
                        nc.tensor.matmul(
                            kcT_ps, mf_t[j][:, cc * 128:(cc + 1) * 128],
                            wsT4[:, j * L:(j + 1) * L],
                            start=(j == 0), stop=(j == 3))
                    nc.scalar.activation(
                        kcT_sb[:, cc * L:(cc + 1) * L],
                        kcT_ps, AF.Identity, bias=c0_col[:, cc:cc + 1])

                # w_allocT
                waT_sb = spool.tile([128, 4 * L], bf16, tag="waT_sb")
                st[s]["waT"] = waT_sb
                for j in range(4):
                    waT_ps = scr[0:128, 448:448 + L]
                    nc.tensor.matmul(waT_ps, wt_sb[:, j * 128:(j + 1) * 128],
                                     PT_sb[:], start=True, stop=True)
                    nc.scalar.copy(waT_sb[:, j * L:(j + 1) * L], waT_ps)

            def prologue_stats(s, cc):
                if "st_col" not in st[s]:
                    st[s]["st_col"] = spool.tile([128, 8], f32, tag="st_col", name="st_col")
                st_col = st[s]["st_col"]
                h_t = hts[s]
                st6 = spool.tile([128, 48], f32, tag="st6")
                for k in range(8):
                    nc.vector.bn_stats(
                        st6[:, k * 6:(k + 1) * 6],
                        h_t[cc][:, k * 512:(k + 1) * 512].bitcast(f32))
                mv = spool.tile([128, 2], f32, tag="mv")
                nc.vector.bn_aggr(mv[:], st6[:])
                sd = spool.tile([128, 1], f32, tag="sd")
                nc.scalar.activation(sd[:], mv[:, 1:2], AF.Sqrt, bias=epz[:, 0:1])
                rs = spool.tile([128, 1], f32, tag="rs")
                nc.vector.reciprocal(rs[:], sd[:])
                nc.vector.tensor_tensor(
                    st_col[:, cc:cc + 1], rs[:], inw_col[:, cc:cc + 1],
                    ALU.mult)
                ms = spool.tile([128, 1], f32, tag="ms")
                nc.vector.tensor_tensor(ms[:], mv[:, 0:1],
                                        st_col[:, cc:cc + 1], ALU.mult)
                nc.vector.tensor_tensor(st_col[:, 4 + cc:5 + cc],
                                        inb_col[:, cc:cc + 1], ms[:],
                                        ALU.subtract)

            def prologue_fold(s):
                scr, st_col, waT_sb = st[s]["scr"], st[s]["st_col"], st[s]["waT"]
                # t2 broadcast to [64,512]
                stT_ps = scr[0:8, 224:352]
                nc.tensor.transpose(stT_ps, st_col[:], id128[:])
                st8r = spool.tile([8, 128], f32r, tag="st8r")
                nc.scalar.copy(st8r[:], stT_ps)
                t2m_t = ps_ring.tile([128, 512], f32, tag="ring")
                t2m_ps = t2m_t[0:L, :]
                for j in range(4):
                    nc.tensor.matmul(t2m_ps[:, j * 128:(j + 1) * 128],
                                     sel8[:, (4 + j) * L:(5 + j) * L], st8r[:],
                                     start=True, stop=True)
                t2m_sb = spool.tile([L, C], bf16, tag="t2m_sb")
                nc.scalar.copy(t2m_sb[:], t2m_ps)

                # gamma then beta, sequentially through scr[0:64, :]
                for j in range(4):
                    nc.tensor.matmul(
                        scr[0:L, :], waT_sb[:, j * L:(j + 1) * L],
                        fw_t[j][:, C:2 * C], start=(j == 0), stop=False)
                nc.tensor.matmul(scr[0:L, :], ones1r[:], fcb_row[:, C:2 * C],
                                 start=False, stop=True)
                gbg = spool.tile([L, C], bf16, tag="gbg")   # gamma
                nc.scalar.copy(gbg[:], scr[0:L, :])
                st[s]["gbg"] = gbg
                for j in range(4):
                    nc.tensor.matmul(
                        scr[0:L, :], waT_sb[:, j * L:(j + 1) * L],
                        fw_t[j][:, 0:C], start=(j == 0), stop=False)
                nc.tensor.matmul(scr[0:L, :], ones1r[:], fcb_row[:, 0:C],
                                 start=False, stop=True)
                # beta2 = beta + t2*gamma
                nc.vector.tensor_tensor(t2m_sb[:], gbg[:], t2m_sb[:], ALU.mult)
                gbb = spool.tile([L, C], bf16, tag="gbb")   # beta2
                nc.vector.tensor_tensor(gbb[:], scr[0:L, :], t2m_sb[:], ALU.add)
                st[s]["gbb"] = gbb
                st[s]["attn_sb"] = attnpool.tile([L, HW], bf16, tag="attn_sb", name="attn_sb")
                st[s]["pend"] = []

            def do_attn(s, pp):
                kcT_sb, h_t, attn_sb = st[s]["kcT"], hts[s], st[s]["attn_sb"]
                attn_t = ps_ring.tile([128, 512], f32, tag="ring")
                attn_ps = attn_t[0:L, :]
                for cc in range(NC4):
                    nc.tensor.matmul(
                        attn_ps, kcT_sb[:, cc * L:(cc + 1) * L],
                        h_t[cc][:, pp * 512:(pp + 1) * 512],
                        start=(cc == 0), stop=(cc == 3))
                nc.scalar.activation(attn_sb[:, pp * 512:(pp + 1) * 512],
                                     attn_ps, AF.Identity, bias=kbs[s][:])

            def finish_piece(s, cc, pp, bm_ps, tmp):
                nc.tensor.matmul(bm_ps[:], id128r[:], tmp[:],
                                 start=False, stop=True)
                outp = piece.tile([128, 512], bf16, tag="outp")
                nc.scalar.copy(outp[:], bm_ps[:])
                nc.sync.dma_start(
                    out_v[s, cc][:, pp * 512:(pp + 1) * 512], outp[:])

            def do_maps(s, pp):
                attn_sb, gbg, gbb = st[s]["attn_sb"], st[s]["gbg"], st[s]["gbb"]
                st_col, h_t, pend = st[s]["st_col"], hts[s], st[s]["pend"]
                aps = attn_sb[:, pp * 512:(pp + 1) * 512]
                for cc in range(NC4):
                    gm_ps = ps_ring.tile([128, 512], f32, tag="ring")
                    bm_ps = ps_ring.tile([128, 512], f32, tag="ring")
                    nc.tensor.matmul(
                        gm_ps[:], gbg[:, cc * 128:(cc + 1) * 128],
                        aps, start=True, stop=True)
                    nc.tensor.matmul(
                        bm_ps[:], gbb[:, cc * 128:(cc + 1) * 128],
                        aps, start=True, stop=False)
                    tmp = piece.tile([128, 512], f32r, tag="tmp")
                    nc.vector.scalar_tensor_tensor(
                        tmp[:], gm_ps[:], st_col[:, cc:cc + 1],
                        h_t[cc][:, pp * 512:(pp + 1) * 512],
                        ALU.mult, ALU.mult)
                    pend.append((s, cc, pp, bm_ps, tmp))
                    if len(pend) > 2:
                        finish_piece(*pend.pop(0))

            # ---- emission schedule: s1 prologue interleaved into s0 pieces --
            prologue_scores(0)
            for cc in range(NC4):
                prologue_stats(0, cc)
            prologue_fold(0)
            for pp in range(NP):
                do_attn(0, pp)
                if pp >= 1:
                    do_maps(0, pp - 1)
                if pp == 3:
                    prologue_scores(1)
                if pp >= 4:
                    prologue_stats(1, pp - 4)
            do_maps(0, NP - 1)
            while st[0]["pend"]:
                finish_piece(*st[0]["pend"].pop(0))
            prologue_fold(1)
            for pp in range(NP):
                do_attn(1, pp)
                if pp >= 1:
                    do_maps(1, pp - 1)
            do_maps(1, NP - 1)
            while st[1]["pend"]:
                finish_piece(*st[1]["pend"].pop(0))

    nc.compile()
    return nc


_NC_CACHE = None


def _get_nc():
    global _NC_CACHE
    if _NC_CACHE is None:
        _NC_CACHE = _build_program()
    return _NC_CACHE


def make_in_maps(inputs):
    import ml_dtypes
    f8 = np.float64
    bfd = ml_dtypes.bfloat16
    h = np.ascontiguousarray(inputs["h"], dtype=np.float32)
    ws = np.asarray(inputs["w_source"], dtype=np.float32)
    wt = np.asarray(inputs["w_target"], dtype=np.float32)
    conv_w = np.asarray(inputs["conv_w"], dtype=np.float32)
    conv_b = np.asarray(inputs["conv_b"], dtype=np.float32)
    fc_k_w = np.asarray(inputs["fc_k_w"], dtype=np.float32)
    fc_k_b = np.asarray(inputs["fc_k_b"], dtype=np.float32)
    fc_w = np.asarray(inputs["fc_w"], dtype=np.float32)
    fc_b = np.asarray(inputs["fc_b"], dtype=np.float32)
    in_w = np.asarray(inputs["in_w"], dtype=np.float32)
    in_b = np.asarray(inputs["in_b"], dtype=np.float32)

    ws_t = np.ascontiguousarray(ws.transpose(0, 2, 1))
    wt_t = np.ascontiguousarray(wt.transpose(0, 2, 1))
    wt_bf = np.ascontiguousarray(wt.astype(bfd))

    # host folds: Mf = fc_k_w.T @ conv_w ; c0 = conv_w.T @ fc_k_b ;
    # kb[b,l] = ws[b] @ (fc_k_w.T @ conv_b) + fc_k_b . conv_b
    cw2 = conv_w[:, :, 0, 0].astype(f8)                     # [Q, C]
    mf = (fc_k_w.astype(f8).T @ cw2).astype(np.float32)     # [D, C]
    c0 = (cw2.T @ fc_k_b.astype(f8)).astype(np.float32)     # [C]
    vb = fc_k_w.astype(f8).T @ conv_b.astype(f8)            # [D]
    kb = (ws.astype(f8) @ vb
          + fc_k_b.astype(f8) @ conv_b.astype(f8)).astype(np.float32)  # [B,L]
    kb_col = kb[:, :, None]                                 # [B,L,1]

    shared = {
        "mf": np.ascontiguousarray(mf),
        "c0_4": np.ascontiguousarray(c0.reshape(4, 128)),
        "fc_w_bf": np.ascontiguousarray(fc_w.T.astype(bfd)),
        "fc_b_row": np.ascontiguousarray(fc_b.reshape(1, 2 * C)),
        "in_w4": np.ascontiguousarray(in_w.reshape(4, 128)),
        "in_b4": np.ascontiguousarray(in_b.reshape(4, 128)),
        "identity64_bf": np.eye(L, dtype=bfd),
        "negid64": -np.eye(L, dtype=np.float32),
        "ones64": np.ones((L, L), dtype=np.float32),
        "eps_zero": np.array([[EPS] * 128, [0.0] * 128], dtype=np.float32),
        "identity128": np.eye(128, dtype=np.float32),
        "identity128_bf": np.eye(128, dtype=bfd),
        "sel8": np.repeat(np.eye(8, dtype=np.float32), L, axis=1),
    }
    in_maps = []
    for i in range(N_CORES):
        lo = i * S
        in_maps.append({
            "h": h[lo:lo + S],
            "ws_t": ws_t[lo:lo + S],
            "wt_t": wt_t[lo:lo + S],
            "wt_bf": wt_bf[lo:lo + S],
            "kb_col": np.ascontiguousarray(kb_col[lo:lo + S]),
            **shared,
        })
    return in_maps


def kernel(**inputs):
    in_maps = make_in_maps(inputs)
    nc = _get_nc()
    res = run_bass_kernel_spmd(nc, in_maps, core_ids=list(range(N_CORES)))
    out = np.concatenate(
        [np.asarray(res.results[i]["out"]) for i in range(N_CORES)], axis=0)
    return out.astype(np.float32).reshape(B, C, H, W)


if __name__ == "__main__":
    rng = np.random.default_rng(0)
    ins = {
        "h": rng.standard_normal((B, C, H, W), dtype=np.float32),
        "w_source": rng.standard_normal((B, L, D), dtype=np.float32),
        "w_target": rng.standard_normal((B, L, D), dtype=np.float32),
        "conv_w": (rng.standard_normal((Q, C, 1, 1), dtype=np.float32)
                   / np.sqrt(C)),
        "conv_b": np.zeros(Q, np.float32),
        "fc_k_w": (rng.standard_normal((Q, D), dtype=np.float32)
                   / np.sqrt(D)),
        "fc_k_b": np.zeros(Q, np.float32),
        "fc_w": (rng.standard_normal((2 * C, D), dtype=np.float32)
                 / np.sqrt(D)),
        "fc_b": np.zeros(2 * C, np.float32),
        "in_w": np.ones(C, np.float32),
        "in_b": np.zeros(C, np.float32),
    }
    out = kernel(**ins)
    print("out", out.shape, out.dtype, float(np.abs(out).max()))
